# revision 30
# baseline (speedup 1.0000x reference)
"""Trainium2 Bass kernel for LucidrainsLFQ (lookup-free quantization).

Reference computation (per token, C=512 channels, D=14 codebook bits):
  y      = W_in @ z_e + b_in                      (project in, 14 dims)
  quant  = sign(y)                                (+-1)
  z_q    = W_out @ quant + b_out                  (project out)
  The softmax over the 2^14 implicit codebook factorizes: the codebook is
  all sign patterns of {-1,+1}^14, so softmax(200 * y . c_j) over j is a
  product of 14 independent Bernoullis.  Split 14 bits = 7 + 7:
     prob[jA, jB] = A[jA] * B[jB]   with A, B 128-way softmaxes
  avg_prob = mean_n A_n outer B_n = (A^T B)/N  -> one 128x128 matmul.
  per-sample entropy = sum_d H_b(sigmoid(400 y_d))  (Bernoulli entropies)
  commit = mean((|y| - 1)^2)
  usage  = distinct hard codes / 16384

Sharding: data-parallel over the 8192 tokens; each of 8 cores handles one
(batch b, 1024-token slice).  Tiny weights/codebook are replicated.  Each
core returns its z_q slice plus partial sums (avg-prob outer product,
entropy/commit sums, hard indices); the host combines the partials.

Performance structure:
  * two 512-token chunks pipelined; z_e arrives in 256 KB per-C-chunk DMAs
    so the first in-projection matmul starts as early as possible; consts
    ride the GpSimd SWDGE ring so they don't serialize behind z_e on SP.
  * dummy matmuls warm the PE clock (HAM) during the initial DMA window.
  * out-projection runs in bf16 (quant is exactly representable; W_out
    rounding contributes ~4e-4 relative on z_q); b_out is folded in as a
    15th contraction row whose activation is constant 1.
  * entropy/commit run post-loop on a PE-transposed [128 tokens, 8, 14]
    layout so the elementwise chain uses all 128 lanes.
"""

import numpy as np

B, C, T = 2, 512, 4096
D = 14
NCORES = 8
TCORE = (B * T) // NCORES  # 1024
CHUNK = 512
NCHUNK = TCORE // CHUNK
NTILE = TCORE // 128
INV_TEMP = 100.0
ENTROPY_W = 0.1
COMMIT_W = 0.25
DIVERSITY_GAMMA = 1.0
EPS = 1e-20

_CACHE = {}


def _build_module():
    import concourse.bacc as bacc
    import concourse.bass as bass
    import concourse.mybir as mybir
    import concourse.tile as tile

    f32 = mybir.dt.float32
    bf16 = mybir.dt.bfloat16
    Act = mybir.ActivationFunctionType
    Alu = mybir.AluOpType
    X = mybir.AxisListType.X
    XY = mybir.AxisListType.XY

    nc = bacc.Bacc("TRN2", target_bir_lowering=False, debug=False,
                   num_devices=NCORES)

    ze_d = nc.dram_tensor("z_part", (C, TCORE), f32, kind="ExternalInput")
    win_d = nc.dram_tensor("w_in_c", (128, 4 * D), f32, kind="ExternalInput")
    wout_d = nc.dram_tensor("w_outT", (D + 1, C), bf16, kind="ExternalInput")
    cst_d = nc.dram_tensor("consts", (D, 273), f32, kind="ExternalInput")
    cbk_d = nc.dram_tensor("cbk_bf", (D, 256), bf16, kind="ExternalInput")
    ones_d = nc.dram_tensor("ones_row", (1, TCORE), bf16, kind="ExternalInput")

    zq_d = nc.dram_tensor("zq_part", (C, TCORE), f32, kind="ExternalOutput")
    avg_d = nc.dram_tensor("avg_part", (128, 128), f32, kind="ExternalOutput")
    stats_d = nc.dram_tensor("stats", (128, 2 * NCHUNK), f32,
                             kind="ExternalOutput")
    idx_d = nc.dram_tensor("idx", (1, TCORE), f32, kind="ExternalOutput")

    ze_r = ze_d.ap().rearrange("(c p) (j t) -> p c j t", p=128, t=CHUNK)
    zq_r = zq_d.ap().rearrange("(c p) (j t) -> p c j t", p=128, t=CHUNK)

    with tile.TileContext(nc) as tc:
        with (
            tc.tile_pool(name="const", bufs=1) as cpool,
            tc.tile_pool(name="data", bufs=1) as dpool,
            tc.tile_pool(name="chunkio", bufs=2) as iopool,
            tc.tile_pool(name="work", bufs=2) as wpool,
            tc.tile_pool(name="ab", bufs=3) as abpool,
            tc.tile_pool(name="ps", bufs=2, space=bass.MemorySpace.PSUM) as pspool,
            tc.tile_pool(name="ps1", bufs=2, space=bass.MemorySpace.PSUM) as ps1pool,
            tc.tile_pool(name="psy", bufs=1, space=bass.MemorySpace.PSUM) as psypool,
            tc.tile_pool(name="psavg", bufs=1, space=bass.MemorySpace.PSUM) as avgpool,
        ):
            # ---- PE warmup: dummy matmuls on a memset tile while z_e lands
            warm_t = cpool.tile([128, 256], f32)
            nc.gpsimd.memset(warm_t[:], 0.125)
            for wi in range(4):
                wp = pspool.tile([128, 256], f32, tag="Lpsum")
                nc.tensor.matmul(wp[:], warm_t[:, 0:128], warm_t[:])

            # ---- input DMAs: z_e chunks on the SP ring, consts on SWDGE
            ze_ts = []
            for j in range(NCHUNK):
                ze_t = iopool.tile([128, 4, CHUNK], f32, tag="ze")
                for c in range(4):
                    nc.sync.dma_start(ze_t[:, c, :], ze_r[:, c, j])
                ze_ts.append(ze_t)

            win_t = cpool.tile([128, 4 * D], f32)
            nc.gpsimd.dma_start(win_t[:], win_d.ap())
            wout_t = cpool.tile([D + 1, C], bf16)
            nc.gpsimd.dma_start(wout_t[:], wout_d.ap())
            cst_t = cpool.tile([D, 273], f32)
            nc.gpsimd.dma_start(cst_t[:], cst_d.ap())
            cbk_t = cpool.tile([D, 256], bf16)
            nc.gpsimd.dma_start(cbk_t[:], cbk_d.ap())

            ones_bf = cpool.tile([D, 1], bf16)
            nc.gpsimd.memset(ones_bf[:], 1.0)

            y_t = dpool.tile([D, TCORE], f32)
            yhi_t = dpool.tile([D, TCORE], bf16)
            ylo_t = dpool.tile([D, TCORE], bf16)
            yT_t = dpool.tile([128, NTILE, D], f32)
            stats_t = dpool.tile([128, 2 * NCHUNK], f32)
            q_t = dpool.tile([D + 1, TCORE], bf16)
            nc.gpsimd.dma_start(q_t[D:D + 1, :], ones_d.ap())
            idx_t = dpool.tile([1, TCORE], f32)
            avg_ps = avgpool.tile([128, 128], f32)

            for j in range(NCHUNK):
                cs = slice(CHUNK * j, CHUNK * (j + 1))
                ze_t = ze_ts[j]

                # ---- project in: y[d, t] = sum_c W_in[d, c] z_e[c, t] ----
                yp = psypool.tile([D, CHUNK], f32, tag="ypsum")
                for c in range(4):
                    nc.tensor.matmul(yp[:], win_t[:, D * c:D * (c + 1)],
                                     ze_t[:, c, :],
                                     start=(c == 0), stop=(c == 3))
                nc.vector.tensor_scalar_add(y_t[:, cs], yp[:],
                                            cst_t[:, 258:259])
                # bf16 hi/lo split of y for single-pass logit matmuls
                nc.scalar.copy(yhi_t[:, cs], y_t[:, cs])
                nc.vector.tensor_tensor(ylo_t[:, cs], y_t[:, cs],
                                        yhi_t[:, cs], op=Alu.subtract)

                # ---- quantize; weighted bits for the hard index ----
                bits_t = wpool.tile([D, CHUNK], f32, tag="bits")
                nc.vector.tensor_scalar(bits_t[:], y_t[:, cs], 0.0, None,
                                        op0=Alu.is_gt)
                nc.vector.tensor_scalar(q_t[0:D, cs], bits_t[:], 2.0, -1.0,
                                        op0=Alu.mult, op1=Alu.add)
                wb_t = wpool.tile([D, CHUNK], bf16, tag="wb")
                nc.vector.tensor_scalar_mul(wb_t[:], bits_t[:],
                                            cst_t[:, 256:257])

                # ---- project out (bf16; b_out folded in as 15th row) ----
                zq_t = iopool.tile([128, 4, CHUNK], f32, tag="zq")
                for c in range(4):
                    zp = pspool.tile([128, CHUNK], f32, tag="zqpsum")
                    nc.tensor.matmul(zp[:], wout_t[:, 128 * c:128 * (c + 1)],
                                     q_t[:, cs])
                    nc.scalar.copy(zq_t[:, c, :], zp[:])
                nc.scalar.dma_start(zq_r[:, :, j], zq_t[:])

                # ---- hard index: sum_d bit_d * 2^(13-d) ----
                ip = ps1pool.tile([1, CHUNK], f32, tag="idxpsum")
                nc.tensor.matmul(ip[:], ones_bf[:], wb_t[:])
                nc.vector.tensor_copy(idx_t[:, cs], ip[:])

                # ---- A/B half-softmaxes; avg_prob outer-product partial ----
                for tt in range(CHUNK // 128):
                    gt = j * (CHUNK // 128) + tt
                    ts = slice(128 * gt, 128 * (gt + 1))
                    Lp = pspool.tile([128, 2, 128], f32, tag="Lpsum")
                    Lpf = Lp[:].rearrange("p a b -> p (a b)")
                    nc.tensor.matmul(Lpf, yhi_t[:, ts], cbk_t[:],
                                     start=True, stop=False)
                    nc.tensor.matmul(Lpf, ylo_t[:, ts], cbk_t[:],
                                     start=False, stop=True)
                    m2 = abpool.tile([128, 2], f32, tag="m2")
                    nc.vector.tensor_reduce(m2[:], Lp[:], axis=X,
                                            op=Alu.max, negate=True)
                    E_t = abpool.tile([128, 2, 128], bf16, tag="E")
                    s2 = abpool.tile([128, 2], f32, tag="s2")
                    for h in range(2):
                        nc.scalar.activation(E_t[:, h, :], Lp[:, h, :], Act.Exp,
                                             bias=m2[:, h:h + 1],
                                             accum_out=s2[:, h:h + 1])
                    sprod = abpool.tile([128, 1], f32, tag="sprod")
                    nc.vector.tensor_tensor(sprod[:], s2[:, 0:1], s2[:, 1:2],
                                            op=Alu.mult)
                    rr = abpool.tile([128, 1], f32, tag="rr")
                    nc.vector.reciprocal(rr[:], sprod[:])
                    Ap_t = abpool.tile([128, 128], bf16, tag="Ap")
                    nc.vector.tensor_scalar_mul(Ap_t[:], E_t[:, 0, :], rr[:])
                    nc.tensor.matmul(avg_ps[:], Ap_t[:], E_t[:, 1, :],
                                     start=(gt == 0), stop=(gt == NTILE - 1))

                # ---- transpose this chunk's y to [128, 4, 14] ----
                for tt in range(CHUNK // 128):
                    gt = j * (CHUNK // 128) + tt
                    tp = ps1pool.tile([128, D], f32, tag="idxpsum")
                    nc.tensor.transpose(tp[:], y_t[:, 128 * gt:128 * (gt + 1)],
                                        cst_t[:, 259:273])
                    if gt % 2 == 0:
                        nc.scalar.copy(yT_t[:, gt, :], tp[:])
                    else:
                        nc.vector.tensor_copy(yT_t[:, gt, :], tp[:])

                # ---- Bernoulli entropy + commitment partial sums ----
                # e = exp(-400|y|);  H_b = ln(1+e) + 400|y| * e/(1+e)
                # ln(1+e) via a degree-6 polynomial on [0,1] (err < 2e-6) so
                # the only ACT function used anywhere is Exp: no table loads.
                LN1P = [0.99988891, -0.49770282, 0.31687717, -0.19223705,
                        0.0841971, -0.01787732]
                yTj = yT_t[:, 4 * j:4 * (j + 1), :]
                sh = [128, CHUNK // 128, D]
                n_t = wpool.tile(sh, f32, tag="n")
                nc.vector.tensor_scalar_mul(n_t[:], yTj, -1.0)
                a_t = wpool.tile(sh, f32, tag="a")
                nc.vector.tensor_tensor(a_t[:], yTj, n_t[:], op=Alu.max)
                e_t = wpool.tile(sh, f32, tag="e")
                nc.scalar.activation(e_t[:], a_t[:], Act.Exp, scale=-400.0)
                d_t = wpool.tile(sh, f32, tag="d")
                nc.vector.tensor_scalar_add(d_t[:], e_t[:], 1.0)
                r_t = wpool.tile(sh, f32, tag="r")
                nc.vector.reciprocal(r_t[:], d_t[:])
                t_t = wpool.tile(sh, f32, tag="t")
                nc.vector.tensor_tensor(t_t[:], e_t[:], r_t[:], op=Alu.mult)
                t2_t = wpool.tile(sh, f32, tag="t2")
                nc.vector.tensor_tensor(t2_t[:], a_t[:], t_t[:], op=Alu.mult)
                h_t = wpool.tile(sh, f32, tag="h")
                nc.vector.tensor_scalar_mul(h_t[:], e_t[:], LN1P[5])
                for k in range(4, -1, -1):
                    nc.vector.scalar_tensor_tensor(h_t[:], h_t[:], LN1P[k],
                                                   e_t[:], op0=Alu.add,
                                                   op1=Alu.mult)
                scr_t = wpool.tile(sh, f32, tag="scr")
                nc.vector.scalar_tensor_tensor(scr_t[:], t2_t[:], 400.0,
                                               h_t[:], op0=Alu.mult,
                                               op1=Alu.add)
                nc.vector.tensor_reduce(stats_t[:, j:j + 1], scr_t[:],
                                        axis=XY, op=Alu.add)
                c1_t = wpool.tile(sh, f32, tag="c1")
                nc.vector.tensor_scalar_add(c1_t[:], a_t[:], -1.0)
                c2_t = wpool.tile(sh, f32, tag="c2")
                nc.vector.tensor_tensor(c2_t[:], c1_t[:], c1_t[:], op=Alu.mult)
                nc.vector.tensor_reduce(stats_t[:, NCHUNK + j:NCHUNK + j + 1],
                                        c2_t[:], axis=XY, op=Alu.add)

            avg_t = wpool.tile([128, 128], f32)
            nc.vector.tensor_copy(avg_t[:], avg_ps[:])
            nc.scalar.dma_start(avg_d.ap(), avg_t[:])
            nc.scalar.dma_start(stats_d.ap(), stats_t[:])
            nc.scalar.dma_start(idx_d.ap(), idx_t[:])

    nc.compile()
    return nc


def _host_consts(b_in):
    # consts layout (14, 273):
    #   [:, 0:256]  block-diagonal scaled half-codebooks:
    #       rows 0:7  cols   0:128 = 200 * C7T ; rows 7:14 cols 128:256 = 200 * C7T
    #   [:, 256] 2^(13-d)   [:, 257] unused   [:, 258] b_in
    #   [:, 259:273] 14x14 identity (for PE transpose)
    c7t = ((((np.arange(128)[None, :] >> np.arange(6, -1, -1)[:, None]) & 1)
            * 2.0 - 1.0)).astype(np.float32)  # (7, 128)
    consts = np.zeros((D, 273), np.float32)
    consts[0:7, 0:128] = 2.0 * INV_TEMP * c7t
    consts[7:14, 128:256] = 2.0 * INV_TEMP * c7t
    consts[:, 256] = (1 << np.arange(D - 1, -1, -1)).astype(np.float32)
    consts[:, 258] = b_in
    consts[:, 259:273] = np.eye(D, dtype=np.float32)
    return consts


def _host_inputs(z_e, W_in, b_in, W_out, b_out):
    import ml_dtypes
    w_in_c = np.ascontiguousarray(
        W_in.T.reshape(4, 128, D).transpose(1, 0, 2).reshape(128, 4 * D))
    w_outT = np.empty((D + 1, C), np.float32)
    w_outT[0:D] = W_out.T
    w_outT[D] = b_out
    w_outT = w_outT.astype(ml_dtypes.bfloat16)
    consts = _host_consts(b_in)
    cbk_bf = consts[:, 0:256].astype(ml_dtypes.bfloat16)
    ones = np.ones((1, TCORE), ml_dtypes.bfloat16)
    in_maps = []
    for k in range(NCORES):
        b, s = divmod(k, NCORES // B)
        zp = np.ascontiguousarray(z_e[b, :, s * TCORE:(s + 1) * TCORE])
        in_maps.append({"z_part": zp, "w_in_c": w_in_c, "w_outT": w_outT,
                        "consts": consts, "cbk_bf": cbk_bf, "ones_row": ones})
    return in_maps


def kernel(z_e, W_in, b_in, W_out, b_out):
    from concourse import bass_utils

    z_e = np.ascontiguousarray(np.asarray(z_e, np.float32))
    W_in = np.asarray(W_in, np.float32)
    b_in = np.asarray(b_in, np.float32)
    W_out = np.asarray(W_out, np.float32)
    b_out = np.asarray(b_out, np.float32)

    if "nc" not in _CACHE:
        _CACHE["nc"] = _build_module()
    nc = _CACHE["nc"]

    in_maps = _host_inputs(z_e, W_in, b_in, W_out, b_out)
    res = bass_utils.run_bass_kernel_spmd(nc, in_maps, core_ids=list(range(NCORES)))
    results = res.results

    z_q = np.empty((B, C, T), np.float32)
    avg_sum = np.zeros((128, 128), np.float64)
    ent_sum = 0.0
    commit_sum = 0.0
    idx_all = []
    for k in range(NCORES):
        b, s = divmod(k, NCORES // B)
        r = results[k]
        z_q[b, :, s * TCORE:(s + 1) * TCORE] = r["zq_part"]
        avg_sum += r["avg_part"].astype(np.float64)
        ent_sum += float(r["stats"][:, 0:NCHUNK].sum(dtype=np.float64))
        commit_sum += float(r["stats"][:, NCHUNK:].sum(dtype=np.float64))
        idx_all.append(r["idx"].ravel())

    n = B * T
    avg_prob = avg_sum / n
    cb_ent = float(-np.sum(avg_prob * np.log(np.clip(avg_prob, EPS, None))))
    ps_ent = ent_sum / n
    commit = commit_sum / (n * D)
    aux = (ps_ent - DIVERSITY_GAMMA * cb_ent) * ENTROPY_W + commit * COMMIT_W

    idx = np.concatenate(idx_all).astype(np.int64)
    usage = len(np.unique(idx)) / 16384.0

    return (z_q, np.float32(aux), np.float32(usage))


# revision 46
# speedup vs baseline: 1.2050x; 1.2050x over previous
"""Trainium2 Bass kernel for LucidrainsLFQ (lookup-free quantization).

Reference computation (per token, C=512 channels, D=14 codebook bits):
  y      = W_in @ z_e + b_in                      (project in, 14 dims)
  quant  = sign(y)                                (+-1)
  z_q    = W_out @ quant + b_out                  (project out)
  The softmax over the 2^14 implicit codebook factorizes: the codebook is
  all sign patterns of {-1,+1}^14, so softmax(200 * y . c_j) over j is a
  product of 14 independent Bernoullis.  Split 14 bits = 7 + 7:
     prob[jA, jB] = A[jA] * B[jB]   with A, B 128-way softmaxes
  avg_prob = mean_n A_n outer B_n = (A^T B)/N  -> one 128x128 matmul.
  per-sample entropy = sum_d H_b(sigmoid(400 y_d))  (Bernoulli entropies)
  commit = mean((|y| - 1)^2)
  usage  = distinct hard codes / 16384

Sharding: data-parallel over the 8192 tokens; each of 8 cores handles one
(batch b, 1024-token slice).  Tiny weights/codebook are replicated.  Each
core returns its z_q slice plus partial sums (avg-prob outer product,
entropy/commit sums, hard indices); the host combines the partials.

Performance structure:
  * four 256-token chunks pipelined (DMA in / project in / quantize /
    project out / DMA out / half-softmaxes); chunk 0's z_e arrives in
    per-C-chunk DMAs so the first matmul starts as early as possible;
    consts ride the GpSimd SWDGE ring so SP's descriptor generation is
    dedicated to z_e.
  * dummy matmuls warm the PE clock (HAM) during the initial DMA window.
  * out-projection runs in bf16 (quant is exactly representable; W_out
    rounding contributes ~2e-3 relative on z_q); b_out is folded in as a
    15th contraction row whose activation is constant 1.  The logit
    matmul uses an exact bf16 hi/lo split of y against a bf16 codebook.
  * entropy/commit run on a PE-transposed [128 tokens, 8, 14] layout,
    with the elementwise chain on the otherwise-idle GpSimd engine using
    only Exp (shared ACT table with the softmax) plus polynomials for
    ln(1+e) and e/(1+e) - so the ACT engine never reloads tables.
"""

import numpy as np

B, C, T = 2, 512, 4096
D = 14
NCORES = 8
TCORE = (B * T) // NCORES  # 1024
CHUNK = 512
NCHUNK = TCORE // CHUNK    # 4
NTILE = TCORE // 128       # 8
TPC = CHUNK // 128         # token-tiles per chunk = 2
INV_TEMP = 100.0
ENTROPY_W = 0.1
COMMIT_W = 0.25
DIVERSITY_GAMMA = 1.0
EPS = 1e-20

# ln(1+x) ~= sum a_k x^k on [0,1] (deg 4, sup err 8e-5)
LN1P4 = [0.99718779, -0.46977512, 0.22310858, -0.05743381]

_CACHE = {}


def _build_module():
    import concourse.bacc as bacc
    import concourse.bass as bass
    import concourse.mybir as mybir
    import concourse.tile as tile

    f32 = mybir.dt.float32
    bf16 = mybir.dt.bfloat16
    Act = mybir.ActivationFunctionType
    Alu = mybir.AluOpType
    X = mybir.AxisListType.X
    XY = mybir.AxisListType.XY

    nc = bacc.Bacc("TRN2", target_bir_lowering=False, debug=False,
                   num_devices=NCORES)

    ze_d = nc.dram_tensor("z_part", (C, TCORE), f32, kind="ExternalInput")
    win_d = nc.dram_tensor("w_in_c", (128, 4 * D), f32, kind="ExternalInput")
    wout_d = nc.dram_tensor("w_outT", (D + 1, C), bf16, kind="ExternalInput")
    cst_d = nc.dram_tensor("consts", (D, 273), f32, kind="ExternalInput")
    cbk_d = nc.dram_tensor("cbk_bf", (D, 256), bf16, kind="ExternalInput")
    ones_d = nc.dram_tensor("ones_row", (1, TCORE), bf16, kind="ExternalInput")

    zq_d = nc.dram_tensor("zq_part", (C, TCORE), f32, kind="ExternalOutput")
    avg_d = nc.dram_tensor("avg_part", (128, 128), f32, kind="ExternalOutput")
    stats_d = nc.dram_tensor("stats", (128, 4), f32, kind="ExternalOutput")
    idx_d = nc.dram_tensor("idx", (1, TCORE), f32, kind="ExternalOutput")

    ze_r = ze_d.ap().rearrange("(c p) (j t) -> p c j t", p=128, t=CHUNK)
    zq_r = zq_d.ap().rearrange("(c p) (j t) -> p c j t", p=128, t=CHUNK)

    with tile.TileContext(nc) as tc:
        with (
            tc.tile_pool(name="const", bufs=1) as cpool,
            tc.tile_pool(name="data", bufs=1) as dpool,
            tc.tile_pool(name="chunkio", bufs=3) as iopool,
            tc.tile_pool(name="work", bufs=2) as wpool,
            tc.tile_pool(name="ab", bufs=3) as abpool,
            tc.tile_pool(name="ps", bufs=2, space=bass.MemorySpace.PSUM) as pspool,
            tc.tile_pool(name="ps1", bufs=1, space=bass.MemorySpace.PSUM) as ps1pool,
            tc.tile_pool(name="psy", bufs=2, space=bass.MemorySpace.PSUM) as psypool,
            tc.tile_pool(name="psavg", bufs=1, space=bass.MemorySpace.PSUM) as avgpool,
        ):
            # ---- PE warmup: dummy matmuls on a memset tile while z_e lands
            warm_t = cpool.tile([128, 256], f32)
            nc.gpsimd.memset(warm_t[:], 0.125)
            for wi in range(4):
                wp = pspool.tile([128, 2, 128], f32, tag="Lpsum")
                nc.tensor.matmul(wp[:].rearrange("p a b -> p (a b)"),
                                 warm_t[:, 0:128], warm_t[:])

            # ---- input DMAs: z_e chunks on the SP ring, consts on SWDGE
            ze_ts = []
            for j in range(NCHUNK):
                ze_t = iopool.tile([128, 4, CHUNK], f32, tag="ze")
                for c in range(4):
                    nc.sync.dma_start(ze_t[:, c, :], ze_r[:, c, j])
                ze_ts.append(ze_t)

            win_t = cpool.tile([128, 4 * D], f32)
            nc.gpsimd.dma_start(win_t[:], win_d.ap())
            wout_t = cpool.tile([D + 1, C], bf16)
            nc.gpsimd.dma_start(wout_t[:], wout_d.ap())
            cst_t = cpool.tile([D, 273], f32)
            nc.gpsimd.dma_start(cst_t[:], cst_d.ap())
            cbk_t = cpool.tile([D, 256], bf16)
            nc.gpsimd.dma_start(cbk_t[:], cbk_d.ap())

            ones_bf = cpool.tile([D, 1], bf16)
            nc.gpsimd.memset(ones_bf[:], 1.0)

            y_t = dpool.tile([D, TCORE], f32)
            yhi_t = dpool.tile([D, TCORE], bf16)
            ylo_t = dpool.tile([D, TCORE], bf16)
            yT_t = dpool.tile([128, NTILE, D], f32)
            stats_t = dpool.tile([128, 4], f32)
            q_t = dpool.tile([D + 1, TCORE], bf16)
            nc.gpsimd.dma_start(q_t[D:D + 1, :], ones_d.ap())
            idx_t = dpool.tile([1, TCORE], f32)
            avg_ps = avgpool.tile([128, 128], f32)

            for j in range(NCHUNK):
                cs = slice(CHUNK * j, CHUNK * (j + 1))
                ze_t = ze_ts[j]

                # ---- project in ----
                yp = psypool.tile([D, CHUNK], f32, tag="ypsum")
                for c in range(4):
                    nc.tensor.matmul(yp[:], win_t[:, D * c:D * (c + 1)],
                                     ze_t[:, c, :],
                                     start=(c == 0), stop=(c == 3))
                nc.vector.tensor_scalar_add(y_t[:, cs], yp[:],
                                            cst_t[:, 258:259])
                # bf16 hi/lo split of y for single-pass logit matmuls
                nc.scalar.copy(yhi_t[:, cs], y_t[:, cs])
                nc.vector.tensor_tensor(ylo_t[:, cs], y_t[:, cs],
                                        yhi_t[:, cs], op=Alu.subtract)

                # ---- transpose this chunk's y to [128, ., 14] ----
                for tt in range(TPC):
                    gt = j * TPC + tt
                    tp = ps1pool.tile([128, D], f32, tag="misc")
                    nc.tensor.transpose(tp[:], y_t[:, 128 * gt:128 * (gt + 1)],
                                        cst_t[:, 259:273])
                    if gt % 2 == 0:
                        nc.scalar.copy(yT_t[:, gt, :], tp[:])
                    else:
                        nc.vector.tensor_copy(yT_t[:, gt, :], tp[:])

                # ---- quantize; weighted bits for the hard index ----
                # wb = (y>0) * 2^(13-d);  quant = wb * 2^(d-12) - 1  (+-1)
                wb_t = wpool.tile([D, CHUNK], bf16, tag="wb")
                nc.vector.tensor_scalar(wb_t[:], y_t[:, cs], 0.0,
                                        cst_t[:, 256:257],
                                        op0=Alu.is_gt, op1=Alu.mult)
                nc.vector.tensor_scalar(q_t[0:D, cs], wb_t[:],
                                        cst_t[:, 257:258], -1.0,
                                        op0=Alu.mult, op1=Alu.add)

                # ---- project out (bf16; b_out folded in as 15th row) ----
                zq_t = iopool.tile([128, 4, CHUNK], f32, tag="zq")
                for c in range(4):
                    zp = pspool.tile([128, CHUNK], f32, tag="zqpsum")
                    nc.tensor.matmul(zp[:], wout_t[:, 128 * c:128 * (c + 1)],
                                     q_t[:, cs])
                    nc.scalar.copy(zq_t[:, c, :], zp[:])
                nc.scalar.dma_start(zq_r[:, :, j], zq_t[:])

                # ---- hard index: sum_d bit_d * 2^(13-d) ----
                ip = ps1pool.tile([1, CHUNK], f32, tag="misc")
                nc.tensor.matmul(ip[:], ones_bf[:], wb_t[:])
                nc.vector.tensor_copy(idx_t[:, cs], ip[:])

                # ---- A/B half-softmaxes; avg_prob outer-product partial ----
                for tt in range(TPC):
                    gt = j * TPC + tt
                    ts = slice(128 * gt, 128 * (gt + 1))
                    Lp = pspool.tile([128, 2, 128], f32, tag="Lpsum")
                    Lpf = Lp[:].rearrange("p a b -> p (a b)")
                    nc.tensor.matmul(Lpf, yhi_t[:, ts], cbk_t[:],
                                     start=True, stop=False)
                    nc.tensor.matmul(Lpf, ylo_t[:, ts], cbk_t[:],
                                     start=False, stop=True)
                    m2 = abpool.tile([128, 2], f32, tag="m2")
                    nc.vector.tensor_reduce(m2[:], Lp[:], axis=X,
                                            op=Alu.max, negate=True)
                    E_t = abpool.tile([128, 2, 128], bf16, tag="E")
                    s2 = abpool.tile([128, 2], f32, tag="s2")
                    for h in range(2):
                        nc.scalar.activation(E_t[:, h, :], Lp[:, h, :], Act.Exp,
                                             bias=m2[:, h:h + 1],
                                             accum_out=s2[:, h:h + 1])
                    sprod = abpool.tile([128, 1], f32, tag="sprod")
                    nc.vector.tensor_tensor(sprod[:], s2[:, 0:1], s2[:, 1:2],
                                            op=Alu.mult)
                    rr = abpool.tile([128, 1], f32, tag="rr")
                    nc.vector.reciprocal(rr[:], sprod[:])
                    Ap_t = abpool.tile([128, 128], bf16, tag="Ap")
                    nc.vector.tensor_scalar_mul(Ap_t[:], E_t[:, 0, :], rr[:])
                    nc.tensor.matmul(avg_ps[:], Ap_t[:], E_t[:, 1, :],
                                     start=(gt == 0), stop=(gt == NTILE - 1))

                # ---- entropy + commit partial sums (per chunk) ----
                # e = exp(-400|y|); H_b = ln(1+e) + 400|y| e/(1+e);
                # ln(1+e) via a deg-4 polynomial so only the Exp ACT table
                # is ever used (no Ln/Sigmoid table loads).
                if True:
                    pr = j
                    yTj = yT_t[:, TPC * pr:TPC * (pr + 1), :]
                    sh = [128, TPC, D]
                    n_t = wpool.tile(sh, f32, tag="n")
                    nc.vector.tensor_scalar_mul(n_t[:], yTj, -1.0)
                    a_t = wpool.tile(sh, f32, tag="a")
                    nc.vector.tensor_tensor(a_t[:], yTj, n_t[:], op=Alu.max)
                    e_t = wpool.tile(sh, f32, tag="e")
                    nc.scalar.activation(e_t[:], a_t[:], Act.Exp, scale=-400.0)
                    d_t = wpool.tile(sh, f32, tag="d")
                    nc.vector.tensor_scalar_add(d_t[:], e_t[:], 1.0)
                    r_t = wpool.tile(sh, f32, tag="r")
                    nc.vector.reciprocal(r_t[:], d_t[:])
                    t2_t = wpool.tile(sh, f32, tag="t2")
                    nc.vector.tensor_tensor(t2_t[:], a_t[:], e_t[:],
                                            op=Alu.mult)
                    t3_t = wpool.tile(sh, f32, tag="t3")
                    nc.vector.tensor_tensor(t3_t[:], t2_t[:], r_t[:],
                                            op=Alu.mult)
                    h_t = wpool.tile(sh, f32, tag="h")
                    nc.vector.tensor_scalar_mul(h_t[:], e_t[:], LN1P4[3])
                    for k in range(2, -1, -1):
                        nc.vector.scalar_tensor_tensor(h_t[:], h_t[:],
                                                       LN1P4[k], e_t[:],
                                                       op0=Alu.add,
                                                       op1=Alu.mult)
                    scr_t = wpool.tile(sh, f32, tag="scr")
                    nc.vector.scalar_tensor_tensor(scr_t[:], t3_t[:], 400.0,
                                                   h_t[:], op0=Alu.mult,
                                                   op1=Alu.add)
                    nc.vector.tensor_reduce(stats_t[:, pr:pr + 1], scr_t[:],
                                            axis=XY, op=Alu.add)
                    c1_t = wpool.tile(sh, f32, tag="c1")
                    nc.vector.tensor_scalar_add(c1_t[:], a_t[:], -1.0)
                    c2_t = wpool.tile(sh, f32, tag="c2")
                    nc.vector.tensor_tensor(c2_t[:], c1_t[:], c1_t[:],
                                            op=Alu.mult)
                    nc.vector.tensor_reduce(stats_t[:, 2 + pr:3 + pr],
                                            c2_t[:], axis=XY, op=Alu.add)

            avg_t = wpool.tile([128, 128], f32)
            nc.vector.tensor_copy(avg_t[:], avg_ps[:])
            nc.scalar.dma_start(avg_d.ap(), avg_t[:])
            nc.scalar.dma_start(stats_d.ap(), stats_t[:])
            nc.scalar.dma_start(idx_d.ap(), idx_t[:])

    nc.compile()
    return nc


def _host_consts(b_in):
    # consts layout (14, 273):
    #   [:, 0:256]  block-diagonal scaled half-codebooks:
    #       rows 0:7  cols   0:128 = 200 * C7T ; rows 7:14 cols 128:256 = 200 * C7T
    #   [:, 256] 2^(13-d)   [:, 257] unused   [:, 258] b_in
    #   [:, 259:273] 14x14 identity (for PE transpose)
    c7t = ((((np.arange(128)[None, :] >> np.arange(6, -1, -1)[:, None]) & 1)
            * 2.0 - 1.0)).astype(np.float32)  # (7, 128)
    consts = np.zeros((D, 273), np.float32)
    consts[0:7, 0:128] = 2.0 * INV_TEMP * c7t
    consts[7:14, 128:256] = 2.0 * INV_TEMP * c7t
    consts[:, 256] = (1 << np.arange(D - 1, -1, -1)).astype(np.float32)
    consts[:, 257] = 2.0 ** (np.arange(D) - 12)
    consts[:, 258] = b_in
    consts[:, 259:273] = np.eye(D, dtype=np.float32)
    return consts


def _host_inputs(z_e, W_in, b_in, W_out, b_out):
    import ml_dtypes
    w_in_c = np.ascontiguousarray(
        W_in.T.reshape(4, 128, D).transpose(1, 0, 2).reshape(128, 4 * D))
    w_outT = np.empty((D + 1, C), np.float32)
    w_outT[0:D] = W_out.T
    w_outT[D] = b_out
    w_outT = w_outT.astype(ml_dtypes.bfloat16)
    consts = _host_consts(b_in)
    cbk_bf = consts[:, 0:256].astype(ml_dtypes.bfloat16)
    ones = np.ones((1, TCORE), ml_dtypes.bfloat16)
    in_maps = []
    for k in range(NCORES):
        b, s = divmod(k, NCORES // B)
        zp = np.ascontiguousarray(z_e[b, :, s * TCORE:(s + 1) * TCORE])
        in_maps.append({"z_part": zp, "w_in_c": w_in_c, "w_outT": w_outT,
                        "consts": consts, "cbk_bf": cbk_bf, "ones_row": ones})
    return in_maps


def kernel(z_e, W_in, b_in, W_out, b_out):
    from concourse import bass_utils

    z_e = np.ascontiguousarray(np.asarray(z_e, np.float32))
    W_in = np.asarray(W_in, np.float32)
    b_in = np.asarray(b_in, np.float32)
    W_out = np.asarray(W_out, np.float32)
    b_out = np.asarray(b_out, np.float32)

    if "nc" not in _CACHE:
        _CACHE["nc"] = _build_module()
    nc = _CACHE["nc"]

    in_maps = _host_inputs(z_e, W_in, b_in, W_out, b_out)
    res = bass_utils.run_bass_kernel_spmd(nc, in_maps, core_ids=list(range(NCORES)))
    results = res.results

    z_q = np.empty((B, C, T), np.float32)
    avg_sum = np.zeros((128, 128), np.float64)
    ent_sum = 0.0
    commit_sum = 0.0
    idx_all = []
    for k in range(NCORES):
        b, s = divmod(k, NCORES // B)
        r = results[k]
        z_q[b, :, s * TCORE:(s + 1) * TCORE] = r["zq_part"]
        avg_sum += r["avg_part"].astype(np.float64)
        ent_sum += float(r["stats"][:, 0:2].sum(dtype=np.float64))
        commit_sum += float(r["stats"][:, 2:4].sum(dtype=np.float64))
        idx_all.append(r["idx"].ravel())

    n = B * T
    avg_prob = avg_sum / n
    cb_ent = float(-np.sum(avg_prob * np.log(np.clip(avg_prob, EPS, None))))
    ps_ent = ent_sum / n
    commit = commit_sum / (n * D)
    aux = (ps_ent - DIVERSITY_GAMMA * cb_ent) * ENTROPY_W + commit * COMMIT_W

    idx = np.concatenate(idx_all).astype(np.int64)
    usage = len(np.unique(idx)) / 16384.0

    return (z_q, np.float32(aux), np.float32(usage))
